# revision 54
# baseline (speedup 1.0000x reference)
"""LIF spike kernel (T-step leaky integrate-and-fire recurrence) on 8 TRN2 cores.

Reference semantics (per element, thre = tanh(w[c])):
    u_t = TAU * u_{t-1} * (1 - o_{t-1}) + x_t
    o_t = (u_t - thre > 0) ? 1.0 : 0.0

Exact power-of-two rescaling: with x'_t = 4^t x_t (host-side, exact since
TAU = 0.25) and th_t = 4^t thre, the recurrence becomes a plain add:
    u'_t = m'_{t-1} + x'_t ;  m'_t = u'_t * (u'_t <= th_t) ;  o_t = u'_t > th_t
Every fp32 op commutes bit-exactly with the scaling, so spikes match the
reference bit-for-bit.

Engine assignment per step (FD = 2048 free columns per partition; the column
space is split Pool-A [0,PA) / DVE [PA,CM) / Pool-B [CM,FD) so each engine's
recurrence chain stays column-local):
    DVE:  ADD over its own cols (tensor_tensor) and all three STT2 chunks
          m' = (u' <= th_t) * u'  (scalar_tensor_tensor is DVE-only on HW)
    Pool: ADD over Pool-A and Pool-B cols (two gpsimd tensor_tensor ops,
          pipelined against DVE's chunked STT2 to keep the cross-engine
          m' -> u' cycle shorter than the steady-state period)
    ACT:  s_t = Sign(u' - th_t) -> bf16, all columns (t <= 14)
    PE:   packs steps 0..14 into PSUM via on-device-built diag(2^t) matmuls
          on s_t (psum = sum 2^t s_t in [-32767, 32767], copied out as int16;
          host adds 32767 -> bits 1..15 of a u16 bitmask)
    step 15 spikes go out directly as uint8 (u' > th_15), skipping the pack.

Sharding: B=32 split across 8 cores (4 each).  Per-core SBUF layout:
partition p = bp*64 + ch, free f = bf*1024 + hw, with b = bp*2 + bf.
"""

import numpy as np

import concourse.bass as bass
import concourse.mybir as mybir
from concourse.bass_utils import run_bass_kernel_spmd

TAU = 0.25
T, B, C, H, W = 16, 32, 64, 32, 32
N_CORES = 8
B_PER = B // N_CORES  # 4
HWF = H * W  # 1024
P = 128
FD = (B_PER // 2) * HWF  # 2048

# Column layout: Pool-A owns [0, PA), DVE owns [PA, CM), Pool-B owns [CM, FD).
PA = 690
CM = PA + 624
NBANK = 4
BW = 512

XS = 4  # x slots
US = 2  # u slots
SS = 4  # sign slots

_cache = {}
last_results = None


def _build_nc():
    nc = bass.Bass("TRN2", target_bir_lowering=False, debug=False, num_devices=N_CORES)
    f32 = mybir.dt.float32
    bf16 = mybir.dt.bfloat16
    i16 = mybir.dt.int16
    u8 = mybir.dt.uint8
    x_d = nc.dram_tensor("x", [T, P, FD], f32, kind="ExternalInput").ap()
    th_d = nc.dram_tensor("th", [P, 1], f32, kind="ExternalInput").ap()
    o16_d = nc.dram_tensor("o16", [P, FD], i16, kind="ExternalOutput").ap()
    o8_d = nc.dram_tensor("o8", [P, FD], u8, kind="ExternalOutput").ap()

    AT = mybir.AluOpType
    AF = mybir.ActivationFunctionType

    X = nc.alloc_sbuf_tensor("Xb", [P, XS * FD], f32).ap()
    U = nc.alloc_sbuf_tensor("Ub", [P, US * FD], f32).ap()
    M = nc.alloc_sbuf_tensor("Mb", [P, FD], f32).ap()
    S = nc.alloc_sbuf_tensor("Sb", [P, SS * FD], bf16).ap()
    WD = nc.alloc_sbuf_tensor("WDb", [P, T * P], bf16).ap()
    VB = nc.alloc_sbuf_tensor("VBb", [P, T], bf16).ap()  # 2^t per column
    O16 = nc.alloc_sbuf_tensor("O16b", [P, FD], i16).ap()
    O8 = nc.alloc_sbuf_tensor("O8b", [P, FD], u8).ap()
    TH = nc.alloc_sbuf_tensor("THb", [P, 1], f32).ap()
    TH4 = nc.alloc_sbuf_tensor("TH4b", [P, T], f32).ap()  # 4^t * tanh(w)
    NT4 = nc.alloc_sbuf_tensor("NT4b", [P, T], f32).ap()  # -4^t * tanh(w)

    PSL = nc.alloc_psum_tensor("psl", [P, 2 * BW], f32).ap()  # banks 0,1
    PSH = nc.alloc_psum_tensor("psh", [P, 2 * BW], f32).ap()  # banks 2,3
    PS = [PSL[:, :BW], PSL[:, BW:], PSH[:, :BW], PSH[:, BW:]]

    def xsl(t):
        return X[:, (t % XS) * FD : (t % XS + 1) * FD]

    def usl(t):
        # u'_0 lives in the x slot (m'_{-1} == 0 so u'_0 == x'_0)
        if t == 0:
            return xsl(0)
        return U[:, (t % US) * FD : (t % US + 1) * FD]

    def ssl(t):
        return S[:, (t % SS) * FD : (t % SS + 1) * FD]

    def wds(t):
        return WD[:, t * P : (t + 1) * P]

    import contextlib

    with contextlib.ExitStack() as st:
        block = st.enter_context(nc.Block())
        dw = st.enter_context(nc.semaphore("dw"))
        dwd = st.enter_context(nc.semaphore("dwd"))
        th_done = st.enter_context(nc.semaphore("th_done"))
        da = st.enter_context(nc.semaphore("da"))  # DVE ADD(t)/x-read -> t+1
        dmA = st.enter_context(nc.semaphore("dmA"))  # DVE STT2pool_A(t) -> t+1
        dmB = st.enter_context(nc.semaphore("dmB"))  # DVE STT2pool_B(t) -> t+1
        paA = st.enter_context(nc.semaphore("paA"))  # Pool ADD_A(t) -> t
        paB = st.enter_context(nc.semaphore("paB"))  # Pool ADD_B(t) -> t
        ds = st.enter_context(nc.semaphore("ds"))  # ACT Sign(t) -> t+1
        pe = st.enter_context(nc.semaphore("pe"))  # PE step t (<=14) -> t+1
        cpD = st.enter_context(nc.semaphore("cpD"))  # DVE psum->i16 copies (banks 0,1)
        dvu8 = st.enter_context(nc.semaphore("dvu8"))  # DVE o15 u8 ready
        do8 = st.enter_context(nc.semaphore("do8"))

        do16 = st.enter_context(nc.semaphore("do16"))
        dxP = [st.enter_context(nc.semaphore(f"dxP{i}")) for i in range(XS)]
        dxA = [st.enter_context(nc.semaphore(f"dxA{i}")) for i in range(XS)]
        dxB = [st.enter_context(nc.semaphore(f"dxB{i}")) for i in range(XS)]

        @block.sync
        def _(sp):
            for t in range(T):
                if t >= XS:
                    j = t - XS
                    if j == 0:
                        sp.wait_ge(dmA, 1)  # DVE STT2 A(0) read x0
                        sp.wait_ge(dmB, 1)  # DVE STT2 B(0) read x0
                        sp.wait_ge(ds, 1)  # ACT Sign(0) read x0
                    else:
                        sp.wait_ge(da, j)  # DVE ADD(j) read x(j)
                        sp.wait_ge(paB, j)  # Pool ADD_A/B(j) read x(j)
                sp.dma_start(out=xsl(t)[:, :PA], in_=x_d[t][:, :PA]).then_inc(
                    dxP[t % XS], 16
                )
                sp.dma_start(out=xsl(t)[:, PA:CM], in_=x_d[t][:, PA:CM]).then_inc(
                    dxA[t % XS], 16
                )
                sp.dma_start(out=xsl(t)[:, CM:], in_=x_d[t][:, CM:]).then_inc(
                    dxB[t % XS], 16
                )
            # packed bitmask out as each half of the copies completes
            sp.wait_ge(cpD, 1)
            sp.dma_start(out=o16_d[:, : 2 * BW], in_=O16[:, : 2 * BW]).then_inc(do16, 16)
            sp.wait_ge(dvu8, 1)
            sp.dma_start(out=o8_d[:, : 2 * BW], in_=O8[:, : 2 * BW]).then_inc(do16, 16)
            sp.wait_ge(dvu8, 2)
            sp.dma_start(out=o8_d[:, 2 * BW :], in_=O8[:, 2 * BW :]).then_inc(do16, 16)
            sp.wait_ge(do16, 48)

        @block.scalar
        def _(ac):
            ac.dma_start(out=TH, in_=th_d).then_inc(dw, 16)
            ac.wait_ge(dw, 16)
            for t in range(T):
                th4 = ac.activation(
                    TH4[:, t : t + 1], TH, AF.Copy, scale=float(4.0**t)
                )
                if t == T - 1:
                    th4.then_inc(th_done, 1)
            for t in range(T):
                ac.activation(NT4[:, t : t + 1], TH, AF.Copy, scale=-float(4.0**t))
            for t in range(T - 1):
                if t == 0:
                    ac.wait_ge(dxP[0], 16)
                    ac.wait_ge(dxA[0], 16)
                    ac.wait_ge(dxB[0], 16)
                else:
                    ac.wait_ge(da, t)  # DVE ADD(t): u' cols [PA, CM)
                    if t < T - 2:
                        ac.wait_ge(paB, t)  # full row needs both Pool adds
                    else:
                        ac.wait_ge(paA, t)  # lo half needs only Pool-A
                if t >= SS:
                    ac.wait_ge(pe, t - SS + 1)  # PE read this sign slot
                if t < T - 2:
                    ac.activation(
                        ssl(t), usl(t), AF.Sign, bias=NT4[:, t : t + 1]
                    ).then_inc(ds, 1)
                else:
                    # t = 14: two halves so PE/copies can start on the lo half
                    ac.activation(
                        ssl(t)[:, : 2 * BW], usl(t)[:, : 2 * BW], AF.Sign,
                        bias=NT4[:, t : t + 1],
                    ).then_inc(ds, 1)
                    ac.wait_ge(paB, t)  # hi half needs Pool-B's add
                    ac.activation(
                        ssl(t)[:, 2 * BW :], usl(t)[:, 2 * BW :], AF.Sign,
                        bias=NT4[:, t : t + 1],
                    ).then_inc(ds, 1)
            ac.wait_ge(pe, 16)  # banks 2,3 accumulation complete
            ac.activation(O16[:, 2 * BW :], PSH, AF.Copy)
            ac.drain()  # own-engine copies must land before the o16-hi DMA reads them
            ac.dma_start(out=o16_d[:, 2 * BW :], in_=O16[:, 2 * BW :]).then_inc(do8, 16)
            ac.wait_ge(do8, 16)


        @block.vector
        def _(dv):
            dv.wait_ge(th_done, 1)
            for t in range(T):
                dv.wait_ge(dxA[t % XS], 16 * (t // XS + 1))
                if t == T - 1:
                    # drain the lo pack before the last add: its DMA is the
                    # longest pole of the tail
                    dv.wait_ge(pe, 15)
                    dv.tensor_scalar(
                        O16[:, : 2 * BW], PSL, 0.0, None, AT.add
                    ).then_inc(cpD, 1)
                if t > 0:
                    if t >= US:
                        dv.wait_ge(ds, t - 1)  # Sign(t-2) read the u slot
                    dv.tensor_tensor(
                        usl(t)[:, PA:CM], M[:, PA:CM], xsl(t)[:, PA:CM], AT.add
                    ).then_inc(da, 1)
                if t < T - 1:
                    # STT2pool_A first: unblocks Pool ADD_A(t+1) early
                    if t > 0:
                        dv.wait_ge(paA, t)  # u' cols [0,PA) = Pool ADD_A(t)
                    else:
                        dv.wait_ge(dxP[0], 16)  # pA-STT2(0) reads x0 cols [0,PA)
                    dv.scalar_tensor_tensor(
                        M[:, :PA],
                        usl(t)[:, :PA],
                        TH4[:, t : t + 1],
                        usl(t)[:, :PA],
                        AT.is_le,
                        AT.mult,
                    ).then_inc(dmA, 1)
                    dv.scalar_tensor_tensor(
                        M[:, PA:CM],
                        usl(t)[:, PA:CM],
                        TH4[:, t : t + 1],
                        usl(t)[:, PA:CM],
                        AT.is_le,
                        AT.mult,
                    )
                    if t > 0:
                        dv.wait_ge(paB, t)  # u' cols [CM,FD) = Pool ADD_B(t)
                    else:
                        dv.wait_ge(dxB[0], 16)  # pB-STT2(0) reads x0's B half
                    dv.scalar_tensor_tensor(
                        M[:, CM:],
                        usl(t)[:, CM:],
                        TH4[:, t : t + 1],
                        usl(t)[:, CM:],
                        AT.is_le,
                        AT.mult,
                    ).then_inc(dmB, 1)
            # o15 spike planes from u'(15)
            dv.wait_ge(paA, T - 1)
            dv.tensor_scalar(
                O8[:, : 2 * BW],
                usl(T - 1)[:, : 2 * BW],
                TH4[:, T - 1 : T],
                None,
                AT.is_gt,
            ).then_inc(dvu8, 1)
            dv.wait_ge(paB, T - 1)  # u' cols [CM,FD) = Pool ADD_B(15)
            dv.tensor_scalar(
                O8[:, 2 * BW :],
                usl(T - 1)[:, 2 * BW :],
                TH4[:, T - 1 : T],
                None,
                AT.is_gt,
            ).then_inc(dvu8, 1)


        @block.gpsimd
        def _(pl):
            # build the pack weights diag(2^t) on-device while Pool is idle
            for t in range(T - 1):
                pl.memset(VB[:, t : t + 1], float(2.0**t))
            for t in range(T - 1):
                asel = pl.affine_select(
                    wds(t),
                    VB[:, t : t + 1].to_broadcast((P, P)),
                    [[1, P]],
                    AT.is_equal,
                    0.0,
                    base=0,
                    channel_multiplier=-1,
                )
                if t == T - 2:
                    asel.then_inc(dwd, 16)
            for t in range(1, T):
                pl.wait_ge(dxP[t % XS], 16 * (t // XS + 1))
                pl.wait_ge(dmA, t)  # m' cols [0,PA) from DVE STT2pool_A(t-1)
                if t >= US:
                    pl.wait_ge(ds, t - 1)  # Sign(t-2) read the u slot
                pl.tensor_tensor(
                    usl(t)[:, :PA], M[:, :PA], xsl(t)[:, :PA], AT.add
                ).then_inc(paA, 1)
                pl.wait_ge(dxB[t % XS], 16 * (t // XS + 1))
                pl.wait_ge(dmB, t)  # m' cols [CM,FD) from DVE STT2pool_B(t-1)
                pl.tensor_tensor(
                    usl(t)[:, CM:], M[:, CM:], xsl(t)[:, CM:], AT.add
                ).then_inc(paB, 1)


        @block.tensor
        def _(pe_eng):
            pe_eng.wait_ge(dwd, 16)
            for t in range(T - 1):
                start = t == 0
                stop = t == T - 2
                pe_eng.wait_ge(ds, t + 1)
                for b in range(NBANK):
                    if t == T - 2 and b == 2:
                        pe_eng.wait_ge(ds, 16)  # hi half of Sign(14)
                    mm = pe_eng.matmul(
                        PS[b],
                        wds(t),
                        ssl(t)[:, b * BW : (b + 1) * BW],
                        start=start,
                        stop=stop,
                        skip_group_check=True,
                    )
                    if t == T - 2 and b == 1:
                        mm.then_inc(pe, 1)  # banks 0,1 stopped -> lo copies can go
                    if b == NBANK - 1:
                        mm.then_inc(pe, 1)

    return nc


def _get_nc():
    if "nc" not in _cache:
        _cache["nc"] = _build_nc()
    return _cache["nc"]


def _shard_x(x):
    """x [T,B,C,H,W] fp32 -> list of 8 contiguous 4^t-prescaled [T,128,2048]."""
    scale = (4.0 ** np.arange(T, dtype=np.float64)).astype(np.float32)
    xf = x.reshape(T, B, C, HWF) * scale[:, None, None, None]
    shards = []
    for i in range(N_CORES):
        xc = xf[:, i * B_PER : (i + 1) * B_PER]  # [T,4,C,1024]
        xc = xc.reshape(T, 2, 2, C, HWF).transpose(0, 1, 3, 2, 4)  # t,bp,c,bf,f
        shards.append(np.ascontiguousarray(xc).reshape(T, P, FD))
    return shards


def _unshard_o(per_core16, per_core8):
    """8x [128,2048] i16 (sum 2^t s_t; bits 1..15 of +32767 = steps 0..14)
    + 8x [128,2048] u8 (step 15). -> [T,B,C,H,W] fp32."""
    outs = []
    for oc16, oc8 in zip(per_core16, per_core8):
        packed = (oc16.astype(np.int32) + 32767).astype(np.uint16)
        shifts = np.arange(1, T, dtype=np.uint16)[:, None, None]
        bits = ((packed[None, :, :] >> shifts) & 1).astype(np.uint8)  # steps 0..14
        allbits = np.concatenate([bits, oc8[None, :, :]], axis=0)  # [T,128,2048]
        ob = allbits.astype(np.float32).reshape(T, 2, C, 2, HWF)
        outs.append(ob.transpose(0, 1, 3, 2, 4).reshape(T, B_PER, C, H, W))
    return np.concatenate(outs, axis=1)


def kernel(x, w):
    global last_results
    x = np.ascontiguousarray(np.asarray(x), dtype=np.float32)
    # tanh on host: bit-identical to the reference's jnp.tanh (verified)
    th = np.tanh(np.asarray(w, dtype=np.float32)).astype(np.float32)
    th = np.tile(th.reshape(64, 1), (2, 1))  # [128,1]

    nc = _get_nc()
    shards = _shard_x(x)
    in_maps = [{"x": shards[i], "th": th} for i in range(N_CORES)]
    last_results = run_bass_kernel_spmd(nc, in_maps, core_ids=list(range(N_CORES)))
    return _unshard_o(
        [last_results.results[i]["o16"] for i in range(N_CORES)],
        [last_results.results[i]["o8"] for i in range(N_CORES)],
    )

